# revision 12
# baseline (speedup 1.0000x reference)
"""Causal MHA (GQA 16q/4kv, QK-RMSnorm, RoPE, tanh softcap 50) on 8 TRN2 cores.

Sharding: 8 shards = (batch b in {0,1}) x (kv-group g in {0..3}).
Each core handles one batch's one kv-head group: 4 Q heads + 1 KV head,
w_q/w_k/w_v column-sharded, w_o row-sharded; host sums the 4 partial
y outputs per batch.

Per-core dataflow (all matmuls via PE, softcap via ACT tanh+exp):
  phase 1: qkv = x @ wqkv (f32r), RMS-norm + RoPE on q/k (DVE),
           v -> fp16 SBUF, q/k transposed to [d, S] via PE.
  phase 2: per q-chunk m (128 rows) and head h:
           scores^raw = qT.T @ kT (f32r, causal extent only)
           t = tanh(raw/400)  (ACT, = tanh(score/50) with score=raw/8)
           p = exp(50 t) fp16 (ACT)  [no max-subtraction needed: |50t|<=7.93]
           mask diagonal chunk (DVE, lower-tri multiply)
           pT chunks via DMA-transpose; o_unnorm/denom = pT.T @ [v|1] (PE)
           o = o_unnorm * recip(denom) (DVE) -> fp16
           y[m] = o @ wo (PE) -> DMA from PSUM to HBM
"""

import numpy as np

D_MODEL = 1024
SEQ = 2048
HD = 64
NQH = 4  # q heads per core
CAP = 50.0
EPS = 1e-5
THETA = 10000.0
P = 128
MC = SEQ // P  # 16 q-chunks
KT = D_MODEL // P  # 8 contraction chunks for projections
N_CORES = 8

_nc_cache = None


def _build_nc():
    import concourse.bass as bass
    import concourse.tile as tile
    from concourse import bacc, mybir
    from concourse.bass import ts
    from concourse.masks import make_identity

    F32 = mybir.dt.float32
    F32R = mybir.dt.float32r
    F16 = mybir.dt.float16
    AF = mybir.ActivationFunctionType
    ALU = mybir.AluOpType
    AX = mybir.AxisListType

    nc = bacc.Bacc("TRN2")
    xT_d = nc.declare_dram_parameter("xT", [D_MODEL, SEQ], F32R, isOutput=False)
    wqkv_d = nc.declare_dram_parameter("wqkv", [D_MODEL, 384], F32R, isOutput=False)
    wo_d = nc.declare_dram_parameter("wo", [256, D_MODEL], F16, isOutput=False)
    cs_d = nc.declare_dram_parameter("cs", [SEQ, 64], F32, isOutput=False)
    tri_d = nc.declare_dram_parameter("tri", [P, P], F16, isOutput=False)
    y_d = nc.declare_dram_parameter("y", [SEQ, D_MODEL], F32, isOutput=True)

    with tile.TileContext(nc) as tc:
        with (
            tc.tile_pool(name="singles", bufs=1) as singles,
            tc.tile_pool(name="psum_s", bufs=2, space="PSUM") as psum_s,
            tc.tile_pool(name="psum_tr", bufs=1, space="PSUM") as psum_tr,
            tc.tile_pool(name="psum_pv", bufs=2, space="PSUM") as psum_pv,
            tc.tile_pool(name="psum_y", bufs=1, space="PSUM") as psum_y,
            tc.tile_pool(name="small", bufs=4) as small,
        ):
            idn32 = singles.tile([P, P], F32)
            make_identity(nc, idn32)
            idn16 = singles.tile([P, P], F16)
            make_identity(nc, idn16)
            tri_sb = singles.tile([P, P], F16)
            nc.sync.dma_start(tri_sb, tri_d[:, :])
            eps_sb = singles.tile([P, 1], F32)
            nc.vector.memset(eps_sb, EPS)
            wo_sb = singles.tile([P, 2, D_MODEL], F16)
            nc.sync.dma_start(wo_sb, wo_d[:, :].rearrange("(o p) n -> p o n", p=P))
            v_sb = singles.tile([P, MC, 65], F16)
            nc.vector.memset(v_sb, 1.0)
            qT_sb = singles.tile([64, NQH, SEQ], F32R)
            kT_sb = singles.tile([64, SEQ], F32R)

            # ---------------- phase 1: projections + norm + rope ----------
            with (
                tc.tile_pool(name="ph1", bufs=1) as ph1,
                tc.tile_pool(name="ptmp", bufs=3) as ptmp,
            ):
                xT_sb = ph1.tile([P, KT, SEQ], F32R)
                wqkv_sb = ph1.tile([P, KT, 384], F32R)
                for kt in range(KT):
                    nc.sync.dma_start(
                        xT_sb[:, kt, :], xT_d[kt * P : (kt + 1) * P, :]
                    )
                    nc.sync.dma_start(
                        wqkv_sb[:, kt, :], wqkv_d[kt * P : (kt + 1) * P, :]
                    )
                cs_sb = ph1.tile([P, MC, 64], F32)
                nc.sync.dma_start(
                    cs_sb, cs_d[:, :].rearrange("(t p) n -> p t n", p=P)
                )

                for m in range(MC):
                    pj_full = psum_s.tile([P, 1024], F32, tag="s", name="pj_full")
                    pj = pj_full[:, 0:384]
                    for kt in range(KT):
                        nc.tensor.matmul(
                            pj,
                            lhsT=xT_sb[:, kt, ts(m, P)],
                            rhs=wqkv_sb[:, kt, :],
                            start=(kt == 0),
                            stop=(kt == KT - 1),
                        )
                    pjh = pj[:, 0:320].rearrange("p (h d) -> p h d", d=HD)
                    sq = ptmp.tile([P, 5, HD], F32, tag="sq")
                    nc.scalar.activation(sq, pjh, AF.Square)
                    ssq = small.tile([P, 5], F32, tag="ssq")
                    nc.vector.reduce_sum(ssq, sq, axis=AX.X)
                    srt = small.tile([P, 5], F32, tag="srt")
                    nc.scalar.activation(srt, ssq, AF.Sqrt, bias=eps_sb, scale=1.0 / HD)
                    rr = small.tile([P, 5], F32, tag="rr")
                    nc.vector.reciprocal(rr, srt)
                    qh = ptmp.tile([P, 5, HD], F32, tag="qh")
                    nc.vector.tensor_mul(
                        qh, pjh, rr[:, :, None].to_broadcast((P, 5, HD))
                    )
                    # v (unnormalized, no rope): cols 320:384
                    nc.vector.tensor_copy(v_sb[:, m, 0:64], pj[:, 320:384])
                    # rope on the 5 q/k heads
                    cosb = cs_sb[:, m, None, 0:32].to_broadcast((P, 5, 32))
                    sinb = cs_sb[:, m, None, 32:64].to_broadcast((P, 5, 32))
                    q1 = qh[:, :, 0:32]
                    q2 = qh[:, :, 32:64]
                    qr = ptmp.tile([P, 5, HD], F32, tag="qr")
                    ta = ptmp.tile([P, 5, 32], F32, tag="ta")
                    tb = ptmp.tile([P, 5, 32], F32, tag="tb")
                    nc.vector.tensor_mul(ta, q1, cosb)
                    nc.vector.tensor_mul(tb, q2, sinb)
                    nc.vector.tensor_tensor(qr[:, :, 0:32], ta, tb, ALU.subtract)
                    tc2 = ptmp.tile([P, 5, 32], F32, tag="tc2")
                    td = ptmp.tile([P, 5, 32], F32, tag="td")
                    nc.vector.tensor_mul(tc2, q2, cosb)
                    nc.vector.tensor_mul(td, q1, sinb)
                    nc.vector.tensor_tensor(qr[:, :, 32:64], tc2, td, ALU.add)
                    # transposes into [d, S] layout
                    for h in range(NQH):
                        tq = psum_tr.tile([P, P], F32, tag="tr")
                        nc.tensor.transpose(tq[0:64, :], qr[:, h, :], idn32)
                        nc.vector.tensor_copy(qT_sb[:, h, ts(m, P)], tq[0:64, :])
                    tk = psum_tr.tile([P, P], F32, tag="tr")
                    nc.tensor.transpose(tk[0:64, :], qr[:, 4, :], idn32)
                    nc.vector.tensor_copy(kT_sb[:, ts(m, P)], tk[0:64, :])

            # ---------------- phase 2: attention --------------------------
            with (
                tc.tile_pool(name="tpool", bufs=1) as tpool,
                tc.tile_pool(name="ppool", bufs=2) as ppool,
                tc.tile_pool(name="ptp", bufs=6) as ptp,
                tc.tile_pool(name="opool", bufs=2) as opool,
                tc.tile_pool(name="otp", bufs=2) as otp,
            ):
                for m in range(MC):
                    km = (m + 1) * P
                    t_all = tpool.tile([P, NQH, SEQ], F32, tag="t")
                    p_all = ppool.tile([P, NQH, SEQ], F16, tag="p")
                    for h in range(NQH):
                        lhsT = qT_sb[:, h, ts(m, P)]
                        for base in range(0, km, 1024):
                            w_sub = min(1024, km - base)
                            pss = psum_s.tile([P, 1024], F32, tag="s")
                            for kb in range(0, w_sub, 512):
                                wb = min(512, w_sub - kb)
                                nc.tensor.matmul(
                                    pss[:, kb : kb + wb],
                                    lhsT=lhsT,
                                    rhs=kT_sb[:, base + kb : base + kb + wb],
                                    start=True,
                                    stop=True,
                                )
                            nc.scalar.activation(
                                t_all[:, h, base : base + w_sub],
                                pss[:, 0:w_sub],
                                AF.Tanh,
                                scale=1.0 / (8.0 * CAP),
                            )
                    nc.scalar.activation(
                        p_all[:, :, 0:km], t_all[:, :, 0:km], AF.Exp, scale=CAP
                    )
                    o_sb = opool.tile([P, NQH, HD], F16, tag="o")
                    for h in range(NQH):
                        # causal mask on the diagonal chunk
                        nc.vector.tensor_mul(
                            p_all[:, h, ts(m, P)], p_all[:, h, ts(m, P)], tri_sb
                        )
                        pv = psum_pv.tile([P, 65], F32, tag="pv")
                        for kc in range(m + 1):
                            pT = ptp.tile([P, P], F16, tag="pT")
                            nc.sync.dma_start_transpose(pT, p_all[:, h, ts(kc, P)])
                            nc.tensor.matmul(
                                pv,
                                lhsT=pT,
                                rhs=v_sb[:, kc, :],
                                start=(kc == 0),
                                stop=(kc == m),
                            )
                        rc = small.tile([P, 1], F32, tag="rc")
                        nc.vector.reciprocal(rc, pv[:, 64:65])
                        nc.vector.tensor_scalar_mul(o_sb[:, h, :], pv[:, 0:64], rc)
                    oT = otp.tile([P, 2, P], F16, tag="oT")
                    for g in range(2):
                        to = psum_tr.tile([P, P], F16, tag="tr")
                        nc.tensor.transpose(to, o_sb[:, 2 * g : 2 * g + 2, :], idn16)
                        nc.vector.tensor_copy(oT[:, g, :], to)
                    y_sb = opool.tile([P, D_MODEL], F32, tag="ysb")
                    for nh in range(2):
                        yp = psum_y.tile([P, 512], F32, tag="y")
                        for g in range(2):
                            nc.tensor.matmul(
                                yp,
                                lhsT=oT[:, g, :],
                                rhs=wo_sb[:, g, ts(nh, 512)],
                                start=(g == 0),
                                stop=(g == 1),
                            )
                        nc.vector.tensor_copy(y_sb[:, ts(nh, 512)], yp)
                    nc.sync.dma_start(y_d[ts(m, P), :], y_sb)
    nc.finalize()
    return nc


def get_nc():
    global _nc_cache
    if _nc_cache is None:
        _nc_cache = _build_nc()
    return _nc_cache


def make_in_maps(x, w_q, w_k, w_v, w_o):
    x = np.asarray(x, np.float32)
    w_q = np.asarray(w_q, np.float32)
    w_k = np.asarray(w_k, np.float32)
    w_v = np.asarray(w_v, np.float32)
    w_o = np.asarray(w_o, np.float32)

    inv_freq = 1.0 / (THETA ** (np.arange(0, HD, 2, dtype=np.float32) / HD))
    freqs = np.arange(SEQ, dtype=np.float32)[:, None] * inv_freq[None, :]
    cs = np.concatenate(
        [np.cos(freqs), np.sin(freqs)], axis=1
    ).astype(np.float32)  # (S, 64)
    tri = np.tril(np.ones((P, P), np.float16))

    in_maps = []
    for c in range(N_CORES):
        b, g = divmod(c, 4)
        wqkv = np.concatenate(
            [
                w_q[:, g * 256 : (g + 1) * 256],
                w_k[:, g * 64 : (g + 1) * 64],
                w_v[:, g * 64 : (g + 1) * 64],
            ],
            axis=1,
        ).astype(np.float32)
        in_maps.append(
            {
                "xT": np.ascontiguousarray(x[b].T),
                "wqkv": np.ascontiguousarray(wqkv),
                "wo": np.ascontiguousarray(
                    w_o[g * 256 : (g + 1) * 256, :]
                ).astype(np.float16),
                "cs": cs,
                "tri": tri,
            }
        )
    return in_maps


def kernel(x, w_q, w_k, w_v, w_o):
    from concourse.bass_utils import run_bass_kernel_spmd

    nc = get_nc()
    in_maps = make_in_maps(x, w_q, w_k, w_v, w_o)
    res = run_bass_kernel_spmd(nc, in_maps, list(range(N_CORES))).results
    y = np.zeros((2, SEQ, D_MODEL), np.float32)
    for c in range(N_CORES):
        y[c // 4] += res[c]["y"]
    return y


# revision 14
# speedup vs baseline: 2.3799x; 2.3799x over previous
"""Causal MHA (GQA 16q/4kv, QK-RMSnorm, RoPE, tanh softcap 50) on 8 TRN2 cores.

Sharding: 8 shards = (batch b in {0,1}) x (kv-group g in {0..3}).
Each core handles one batch's one kv-head group: 4 Q heads + 1 KV head,
w_q/w_k/w_v column-sharded, w_o row-sharded; host sums the 4 partial
y outputs per batch.

Per-core dataflow (all matmuls via PE, softcap via ACT tanh+exp):
  phase 1: qkv = x @ wqkv (f32r), RMS-norm + RoPE on q/k (DVE),
           v -> fp16 SBUF, q/k transposed to [d, S] via PE.
  phase 2: per q-chunk m (128 rows) and head h:
           scores^raw = qT.T @ kT (f32r, causal extent only)
           t = tanh(raw/400)  (ACT, = tanh(score/50) with score=raw/8)
           p = exp(50 t) fp16 (ACT)  [no max-subtraction needed: |50t|<=7.93]
           mask diagonal chunk (DVE, lower-tri multiply)
           pT chunks via DMA-transpose; o_unnorm/denom = pT.T @ [v|1] (PE)
           o = o_unnorm * recip(denom) (DVE) -> fp16
           y[m] = o @ wo (PE) -> DMA from PSUM to HBM
"""

import numpy as np

D_MODEL = 1024
SEQ = 2048
HD = 64
NQH = 4  # q heads per core
CAP = 50.0
EPS = 1e-5
THETA = 10000.0
P = 128
MC = SEQ // P  # 16 q-chunks
KT = D_MODEL // P  # 8 contraction chunks for projections
N_CORES = 8

_nc_cache = None


def _build_nc():
    import concourse.bass as bass
    import concourse.tile as tile
    from concourse import bacc, mybir
    from concourse.bass import ts
    from concourse.masks import make_identity

    F32 = mybir.dt.float32
    F32R = mybir.dt.float32r
    F16 = mybir.dt.float16
    AF = mybir.ActivationFunctionType
    ALU = mybir.AluOpType
    AX = mybir.AxisListType

    nc = bacc.Bacc("TRN2")
    xT_d = nc.declare_dram_parameter("xT", [D_MODEL, SEQ], F32R, isOutput=False)
    wqkv_d = nc.declare_dram_parameter("wqkv", [D_MODEL, 384], F32R, isOutput=False)
    wo_d = nc.declare_dram_parameter("wo", [256, D_MODEL], F16, isOutput=False)
    cs_d = nc.declare_dram_parameter("cs", [SEQ, 64], F32, isOutput=False)
    tri_d = nc.declare_dram_parameter("tri", [P, P], F16, isOutput=False)
    y_d = nc.declare_dram_parameter("y", [SEQ, D_MODEL], F32, isOutput=True)

    with tile.TileContext(nc) as tc:
        with (
            tc.tile_pool(name="singles", bufs=1) as singles,
            tc.tile_pool(name="psum_s", bufs=2, space="PSUM") as psum_s,
            tc.tile_pool(name="psum_tr", bufs=1, space="PSUM") as psum_tr,
            tc.tile_pool(name="psum_pv", bufs=2, space="PSUM") as psum_pv,
            tc.tile_pool(name="psum_y", bufs=1, space="PSUM") as psum_y,
            tc.tile_pool(name="small", bufs=4) as small,
        ):
            idn32 = singles.tile([P, P], F32)
            make_identity(nc, idn32)
            idn16 = singles.tile([P, P], F16)
            make_identity(nc, idn16)
            tri_sb = singles.tile([P, P], F16)
            nc.sync.dma_start(tri_sb, tri_d[:, :])
            eps_sb = singles.tile([P, 1], F32)
            nc.vector.memset(eps_sb, EPS)
            wo_sb = singles.tile([P, 2, D_MODEL], F16)
            nc.sync.dma_start(wo_sb, wo_d[:, :].rearrange("(o p) n -> p o n", p=P))
            v_sb = singles.tile([P, MC, 65], F16)
            nc.vector.memset(v_sb, 1.0)
            qT_sb = singles.tile([64, NQH, SEQ], F32R)
            kT_sb = singles.tile([64, SEQ], F32R)

            # ---------------- phase 1: projections + norm + rope ----------
            with (
                tc.tile_pool(name="ph1", bufs=1) as ph1,
                tc.tile_pool(name="ptmp", bufs=3) as ptmp,
            ):
                xT_sb = ph1.tile([P, KT, SEQ], F32R)
                wqkv_sb = ph1.tile([P, KT, 384], F32R)
                for kt in range(KT):
                    nc.sync.dma_start(
                        xT_sb[:, kt, :], xT_d[kt * P : (kt + 1) * P, :]
                    )
                    nc.sync.dma_start(
                        wqkv_sb[:, kt, :], wqkv_d[kt * P : (kt + 1) * P, :]
                    )
                cs_sb = ph1.tile([P, MC, 64], F32)
                nc.sync.dma_start(
                    cs_sb, cs_d[:, :].rearrange("(t p) n -> p t n", p=P)
                )

                for m in range(MC):
                    pj_full = psum_s.tile([P, 1024], F32, tag="s", name="pj_full")
                    pj = pj_full[:, 0:384]
                    for kt in range(KT):
                        nc.tensor.matmul(
                            pj,
                            lhsT=xT_sb[:, kt, ts(m, P)],
                            rhs=wqkv_sb[:, kt, :],
                            start=(kt == 0),
                            stop=(kt == KT - 1),
                        )
                    pjh = pj[:, 0:320].rearrange("p (h d) -> p h d", d=HD)
                    sq = ptmp.tile([P, 5, HD], F32, tag="sq")
                    nc.scalar.activation(sq, pjh, AF.Square)
                    ssq = small.tile([P, 5], F32, tag="ssq")
                    nc.vector.reduce_sum(ssq, sq, axis=AX.X)
                    srt = small.tile([P, 5], F32, tag="srt")
                    nc.scalar.activation(srt, ssq, AF.Sqrt, bias=eps_sb, scale=1.0 / HD)
                    rr = small.tile([P, 5], F32, tag="rr")
                    nc.vector.reciprocal(rr, srt)
                    qh = ptmp.tile([P, 5, HD], F32, tag="qh")
                    nc.vector.tensor_mul(
                        qh, pjh, rr[:, :, None].to_broadcast((P, 5, HD))
                    )
                    # v (unnormalized, no rope): cols 320:384
                    nc.vector.tensor_copy(v_sb[:, m, 0:64], pj[:, 320:384])
                    # rope on the 5 q/k heads
                    cosb = cs_sb[:, m, None, 0:32].to_broadcast((P, 5, 32))
                    sinb = cs_sb[:, m, None, 32:64].to_broadcast((P, 5, 32))
                    q1 = qh[:, :, 0:32]
                    q2 = qh[:, :, 32:64]
                    qr = ptmp.tile([P, 5, HD], F32, tag="qr")
                    ta = ptmp.tile([P, 5, 32], F32, tag="ta")
                    tb = ptmp.tile([P, 5, 32], F32, tag="tb")
                    nc.vector.tensor_mul(ta, q1, cosb)
                    nc.vector.tensor_mul(tb, q2, sinb)
                    nc.vector.tensor_tensor(qr[:, :, 0:32], ta, tb, ALU.subtract)
                    tc2 = ptmp.tile([P, 5, 32], F32, tag="tc2")
                    td = ptmp.tile([P, 5, 32], F32, tag="td")
                    nc.vector.tensor_mul(tc2, q2, cosb)
                    nc.vector.tensor_mul(td, q1, sinb)
                    nc.vector.tensor_tensor(qr[:, :, 32:64], tc2, td, ALU.add)
                    # transposes into [d, S] layout
                    for h in range(NQH):
                        tq = psum_tr.tile([P, P], F32, tag="tr")
                        nc.tensor.transpose(tq[0:64, :], qr[:, h, :], idn32)
                        nc.vector.tensor_copy(qT_sb[:, h, ts(m, P)], tq[0:64, :])
                    tk = psum_tr.tile([P, P], F32, tag="tr")
                    nc.tensor.transpose(tk[0:64, :], qr[:, 4, :], idn32)
                    nc.vector.tensor_copy(kT_sb[:, ts(m, P)], tk[0:64, :])

            # ---------------- phase 2: attention --------------------------
            with (
                tc.tile_pool(name="tpool", bufs=1) as tpool,
                tc.tile_pool(name="ppool", bufs=2) as ppool,
                tc.tile_pool(name="ptp", bufs=3) as ptp,
                tc.tile_pool(name="opool", bufs=2) as opool,
                tc.tile_pool(name="otp", bufs=2) as otp,
            ):
                for m in range(MC):
                    km = (m + 1) * P
                    t_all = tpool.tile([P, NQH, SEQ], F32, tag="t")
                    p_all = ppool.tile([P, NQH, SEQ], F16, tag="p")
                    for h in range(NQH):
                        lhsT = qT_sb[:, h, ts(m, P)]
                        for base in range(0, km, 1024):
                            w_sub = min(1024, km - base)
                            pss = psum_s.tile([P, 1024], F32, tag="s")
                            for kb in range(0, w_sub, 512):
                                wb = min(512, w_sub - kb)
                                nc.tensor.matmul(
                                    pss[:, kb : kb + wb],
                                    lhsT=lhsT,
                                    rhs=kT_sb[:, base + kb : base + kb + wb],
                                    start=True,
                                    stop=True,
                                )
                            nc.scalar.activation(
                                t_all[:, h, base : base + w_sub],
                                pss[:, 0:w_sub],
                                AF.Tanh,
                                scale=1.0 / (8.0 * CAP),
                            )
                    nc.scalar.activation(
                        p_all[:, :, 0:km], t_all[:, :, 0:km], AF.Exp, scale=CAP
                    )
                    o_sb = opool.tile([P, NQH, HD], F16, tag="o")
                    for h in range(NQH):
                        # causal mask on the diagonal chunk
                        nc.vector.tensor_mul(
                            p_all[:, h, ts(m, P)], p_all[:, h, ts(m, P)], tri_sb
                        )
                        pT = ptp.tile([P, MC, P], F16, tag="pT")
                        nc.sync.dma_start_transpose(
                            pT[:, 0 : m + 1, :], p_all[:, h, 0:km]
                        )
                        pv = psum_pv.tile([P, 65], F32, tag="pv")
                        for kc in range(m + 1):
                            nc.tensor.matmul(
                                pv,
                                lhsT=pT[:, kc, :],
                                rhs=v_sb[:, kc, :],
                                start=(kc == 0),
                                stop=(kc == m),
                            )
                        rc = small.tile([P, 1], F32, tag="rc")
                        nc.vector.reciprocal(rc, pv[:, 64:65])
                        nc.vector.tensor_scalar_mul(o_sb[:, h, :], pv[:, 0:64], rc)
                    oT = otp.tile([P, 2, P], F16, tag="oT")
                    for g in range(2):
                        to = psum_tr.tile([P, P], F16, tag="tr")
                        nc.tensor.transpose(to, o_sb[:, 2 * g : 2 * g + 2, :], idn16)
                        nc.vector.tensor_copy(oT[:, g, :], to)
                    y_sb = opool.tile([P, D_MODEL], F32, tag="ysb")
                    for nh in range(2):
                        yp = psum_y.tile([P, 512], F32, tag="y")
                        for g in range(2):
                            nc.tensor.matmul(
                                yp,
                                lhsT=oT[:, g, :],
                                rhs=wo_sb[:, g, ts(nh, 512)],
                                start=(g == 0),
                                stop=(g == 1),
                            )
                        nc.vector.tensor_copy(y_sb[:, ts(nh, 512)], yp)
                    nc.sync.dma_start(y_d[ts(m, P), :], y_sb)
    nc.finalize()
    return nc


def get_nc():
    global _nc_cache
    if _nc_cache is None:
        _nc_cache = _build_nc()
    return _nc_cache


def make_in_maps(x, w_q, w_k, w_v, w_o):
    x = np.asarray(x, np.float32)
    w_q = np.asarray(w_q, np.float32)
    w_k = np.asarray(w_k, np.float32)
    w_v = np.asarray(w_v, np.float32)
    w_o = np.asarray(w_o, np.float32)

    inv_freq = 1.0 / (THETA ** (np.arange(0, HD, 2, dtype=np.float32) / HD))
    freqs = np.arange(SEQ, dtype=np.float32)[:, None] * inv_freq[None, :]
    cs = np.concatenate(
        [np.cos(freqs), np.sin(freqs)], axis=1
    ).astype(np.float32)  # (S, 64)
    tri = np.tril(np.ones((P, P), np.float16))

    in_maps = []
    for c in range(N_CORES):
        b, g = divmod(c, 4)
        wqkv = np.concatenate(
            [
                w_q[:, g * 256 : (g + 1) * 256],
                w_k[:, g * 64 : (g + 1) * 64],
                w_v[:, g * 64 : (g + 1) * 64],
            ],
            axis=1,
        ).astype(np.float32)
        in_maps.append(
            {
                "xT": np.ascontiguousarray(x[b].T),
                "wqkv": np.ascontiguousarray(wqkv),
                "wo": np.ascontiguousarray(
                    w_o[g * 256 : (g + 1) * 256, :]
                ).astype(np.float16),
                "cs": cs,
                "tri": tri,
            }
        )
    return in_maps


def kernel(x, w_q, w_k, w_v, w_o):
    from concourse.bass_utils import run_bass_kernel_spmd

    nc = get_nc()
    in_maps = make_in_maps(x, w_q, w_k, w_v, w_o)
    res = run_bass_kernel_spmd(nc, in_maps, list(range(N_CORES))).results
    y = np.zeros((2, SEQ, D_MODEL), np.float32)
    for c in range(N_CORES):
        y[c // 4] += res[c]["y"]
    return y


# revision 16
# speedup vs baseline: 2.5046x; 1.0524x over previous
"""Causal MHA (GQA 16q/4kv, QK-RMSnorm, RoPE, tanh softcap 50) on 8 TRN2 cores.

Sharding: 8 shards = (batch b in {0,1}) x (kv-group g in {0..3}).
Each core handles one batch's one kv-head group: 4 Q heads + 1 KV head,
w_q/w_k/w_v column-sharded, w_o row-sharded; host sums the 4 partial
y outputs per batch.

Per-core dataflow (all matmuls via PE, softcap via ACT tanh+exp):
  phase 1: qkv = x @ wqkv (f32r), RMS-norm + RoPE on q/k (DVE),
           v -> fp16 SBUF, q/k transposed to [d, S] via PE.
  phase 2: per q-chunk m (128 rows) and head h:
           scores^raw = qT.T @ kT (f32r, causal extent only)
           t = tanh(raw/400)  (ACT, = tanh(score/50) with score=raw/8)
           p = exp(50 t) fp16 (ACT)  [no max-subtraction needed: |50t|<=7.93]
           mask diagonal chunk (DVE, lower-tri multiply)
           pT chunks via DMA-transpose; o_unnorm/denom = pT.T @ [v|1] (PE)
           o = o_unnorm * recip(denom) (DVE) -> fp16
           y[m] = o @ wo (PE) -> DMA from PSUM to HBM
"""

import numpy as np

D_MODEL = 1024
SEQ = 2048
HD = 64
NQH = 4  # q heads per core
CAP = 50.0
EPS = 1e-5
THETA = 10000.0
P = 128
MC = SEQ // P  # 16 q-chunks
KT = D_MODEL // P  # 8 contraction chunks for projections
N_CORES = 8

_nc_cache = None


def _build_nc():
    import concourse.bass as bass
    import concourse.tile as tile
    from concourse import bacc, mybir
    from concourse.bass import ts
    from concourse.masks import make_identity

    F32 = mybir.dt.float32
    F32R = mybir.dt.float32r
    F16 = mybir.dt.float16
    AF = mybir.ActivationFunctionType
    ALU = mybir.AluOpType
    AX = mybir.AxisListType

    nc = bacc.Bacc("TRN2")
    xT_d = nc.declare_dram_parameter("xT", [D_MODEL, SEQ], F32R, isOutput=False)
    wqkv_d = nc.declare_dram_parameter("wqkv", [D_MODEL, 384], F32R, isOutput=False)
    wo_d = nc.declare_dram_parameter("wo", [256, D_MODEL], F16, isOutput=False)
    cs_d = nc.declare_dram_parameter("cs", [SEQ, 64], F32, isOutput=False)
    tri_d = nc.declare_dram_parameter("tri", [P, P], F16, isOutput=False)
    y_d = nc.declare_dram_parameter("y", [SEQ, D_MODEL], F32, isOutput=True)

    with tile.TileContext(nc) as tc:
        with (
            tc.tile_pool(name="singles", bufs=1) as singles,
            tc.tile_pool(name="psum_s", bufs=2, space="PSUM") as psum_s,
            tc.tile_pool(name="psum_tr", bufs=1, space="PSUM") as psum_tr,
            tc.tile_pool(name="psum_pv", bufs=2, space="PSUM") as psum_pv,
            tc.tile_pool(name="psum_y", bufs=1, space="PSUM") as psum_y,
            tc.tile_pool(name="small", bufs=4) as small,
        ):
            idn32 = singles.tile([P, P], F32)
            make_identity(nc, idn32)
            idn16 = singles.tile([P, P], F16)
            make_identity(nc, idn16)
            tri_sb = singles.tile([P, P], F16)
            nc.sync.dma_start(tri_sb, tri_d[:, :])
            eps_sb = singles.tile([P, 1], F32)
            nc.vector.memset(eps_sb, EPS)
            wo_sb = singles.tile([P, 2, D_MODEL], F16)
            nc.sync.dma_start(wo_sb, wo_d[:, :].rearrange("(o p) n -> p o n", p=P))
            v_sb = singles.tile([P, MC, 65], F16)
            nc.vector.memset(v_sb, 1.0)
            qT_sb = singles.tile([64, NQH, SEQ], F32R)
            kT_sb = singles.tile([64, SEQ], F32R)

            # ---------------- phase 1: projections + norm + rope ----------
            with (
                tc.tile_pool(name="ph1", bufs=1) as ph1,
                tc.tile_pool(name="ptmp", bufs=3) as ptmp,
            ):
                xT_sb = ph1.tile([P, KT, SEQ], F32R)
                wqkv_sb = ph1.tile([P, KT, 384], F32R)
                for kt in range(KT):
                    nc.sync.dma_start(
                        xT_sb[:, kt, :], xT_d[kt * P : (kt + 1) * P, :]
                    )
                    nc.sync.dma_start(
                        wqkv_sb[:, kt, :], wqkv_d[kt * P : (kt + 1) * P, :]
                    )
                cs_sb = ph1.tile([P, MC, 64], F32)
                nc.sync.dma_start(
                    cs_sb, cs_d[:, :].rearrange("(t p) n -> p t n", p=P)
                )

                for m in range(MC):
                    pj_full = psum_s.tile([P, 1024], F32, tag="s", name="pj_full")
                    pj = pj_full[:, 0:384]
                    for kt in range(KT):
                        nc.tensor.matmul(
                            pj,
                            lhsT=xT_sb[:, kt, ts(m, P)],
                            rhs=wqkv_sb[:, kt, :],
                            start=(kt == 0),
                            stop=(kt == KT - 1),
                        )
                    pjh = pj[:, 0:320].rearrange("p (h d) -> p h d", d=HD)
                    sq = ptmp.tile([P, 5, HD], F32, tag="sq")
                    nc.scalar.activation(sq, pjh, AF.Square)
                    ssq = small.tile([P, 5], F32, tag="ssq")
                    nc.vector.reduce_sum(ssq, sq, axis=AX.X)
                    srt = small.tile([P, 5], F32, tag="srt")
                    nc.scalar.activation(srt, ssq, AF.Sqrt, bias=eps_sb, scale=1.0 / HD)
                    rr = small.tile([P, 5], F32, tag="rr")
                    nc.vector.reciprocal(rr, srt)
                    qh = ptmp.tile([P, 5, HD], F32, tag="qh")
                    nc.vector.tensor_mul(
                        qh, pjh, rr[:, :, None].to_broadcast((P, 5, HD))
                    )
                    # v (unnormalized, no rope): cols 320:384
                    nc.vector.tensor_copy(v_sb[:, m, 0:64], pj[:, 320:384])
                    # rope on the 5 q/k heads
                    cosb = cs_sb[:, m, None, 0:32].to_broadcast((P, 5, 32))
                    sinb = cs_sb[:, m, None, 32:64].to_broadcast((P, 5, 32))
                    q1 = qh[:, :, 0:32]
                    q2 = qh[:, :, 32:64]
                    qr = ptmp.tile([P, 5, HD], F32, tag="qr")
                    ta = ptmp.tile([P, 5, 32], F32, tag="ta")
                    tb = ptmp.tile([P, 5, 32], F32, tag="tb")
                    nc.vector.tensor_mul(ta, q1, cosb)
                    nc.vector.tensor_mul(tb, q2, sinb)
                    nc.vector.tensor_tensor(qr[:, :, 0:32], ta, tb, ALU.subtract)
                    tc2 = ptmp.tile([P, 5, 32], F32, tag="tc2")
                    td = ptmp.tile([P, 5, 32], F32, tag="td")
                    nc.vector.tensor_mul(tc2, q2, cosb)
                    nc.vector.tensor_mul(td, q1, sinb)
                    nc.vector.tensor_tensor(qr[:, :, 32:64], tc2, td, ALU.add)
                    # transposes into [d, S] layout
                    for h in range(NQH):
                        tq = psum_tr.tile([P, P], F32, tag="tr")
                        nc.tensor.transpose(tq[0:64, :], qr[:, h, :], idn32)
                        nc.vector.tensor_copy(qT_sb[:, h, ts(m, P)], tq[0:64, :])
                    tk = psum_tr.tile([P, P], F32, tag="tr")
                    nc.tensor.transpose(tk[0:64, :], qr[:, 4, :], idn32)
                    nc.vector.tensor_copy(kT_sb[:, ts(m, P)], tk[0:64, :])

            # ---------------- phase 2: attention --------------------------
            with (
                tc.tile_pool(name="tpool", bufs=2) as tpool,
                tc.tile_pool(name="ppool", bufs=3) as ppool,
                tc.tile_pool(name="ptp", bufs=3) as ptp,
                tc.tile_pool(name="opool", bufs=2) as opool,
                tc.tile_pool(name="otp", bufs=2) as otp,
            ):
                for m in range(MC):
                    km = (m + 1) * P
                    o_sb = opool.tile([P, NQH, HD], F16, tag="o")
                    for h in range(NQH):
                        lhsT = qT_sb[:, h, ts(m, P)]
                        t_h = tpool.tile([P, SEQ], F32, tag="t")
                        for base in range(0, km, 1024):
                            w_sub = min(1024, km - base)
                            pss = psum_s.tile([P, 1024], F32, tag="s")
                            for kb in range(0, w_sub, 512):
                                wb = min(512, w_sub - kb)
                                nc.tensor.matmul(
                                    pss[:, kb : kb + wb],
                                    lhsT=lhsT,
                                    rhs=kT_sb[:, base + kb : base + kb + wb],
                                    start=True,
                                    stop=True,
                                )
                            nc.scalar.activation(
                                t_h[:, base : base + w_sub],
                                pss[:, 0:w_sub],
                                AF.Tanh,
                                scale=1.0 / (8.0 * CAP),
                            )
                        p_h = ppool.tile([P, SEQ], F16, tag="p")
                        nc.scalar.activation(
                            p_h[:, 0:km], t_h[:, 0:km], AF.Exp, scale=CAP
                        )
                        # causal mask on the diagonal chunk
                        nc.vector.tensor_mul(
                            p_h[:, ts(m, P)], p_h[:, ts(m, P)], tri_sb
                        )
                        pT = ptp.tile([P, MC, P], F16, tag="pT")
                        nc.sync.dma_start_transpose(pT[:, 0 : m + 1, :], p_h[:, 0:km])
                        pv = psum_pv.tile([P, 65], F32, tag="pv")
                        for kc in range(m + 1):
                            nc.tensor.matmul(
                                pv,
                                lhsT=pT[:, kc, :],
                                rhs=v_sb[:, kc, :],
                                start=(kc == 0),
                                stop=(kc == m),
                            )
                        rc = small.tile([P, 1], F32, tag="rc")
                        nc.vector.reciprocal(rc, pv[:, 64:65])
                        nc.vector.tensor_scalar_mul(o_sb[:, h, :], pv[:, 0:64], rc)
                    oT = otp.tile([P, 2, P], F16, tag="oT")
                    for g in range(2):
                        to = psum_tr.tile([P, P], F16, tag="tr")
                        nc.tensor.transpose(to, o_sb[:, 2 * g : 2 * g + 2, :], idn16)
                        nc.vector.tensor_copy(oT[:, g, :], to)
                    y_sb = opool.tile([P, D_MODEL], F32, tag="ysb")
                    for nh in range(2):
                        yp = psum_y.tile([P, 512], F32, tag="y")
                        for g in range(2):
                            nc.tensor.matmul(
                                yp,
                                lhsT=oT[:, g, :],
                                rhs=wo_sb[:, g, ts(nh, 512)],
                                start=(g == 0),
                                stop=(g == 1),
                            )
                        nc.vector.tensor_copy(y_sb[:, ts(nh, 512)], yp)
                    nc.sync.dma_start(y_d[ts(m, P), :], y_sb)
    nc.finalize()
    return nc


def get_nc():
    global _nc_cache
    if _nc_cache is None:
        _nc_cache = _build_nc()
    return _nc_cache


def make_in_maps(x, w_q, w_k, w_v, w_o):
    x = np.asarray(x, np.float32)
    w_q = np.asarray(w_q, np.float32)
    w_k = np.asarray(w_k, np.float32)
    w_v = np.asarray(w_v, np.float32)
    w_o = np.asarray(w_o, np.float32)

    inv_freq = 1.0 / (THETA ** (np.arange(0, HD, 2, dtype=np.float32) / HD))
    freqs = np.arange(SEQ, dtype=np.float32)[:, None] * inv_freq[None, :]
    cs = np.concatenate(
        [np.cos(freqs), np.sin(freqs)], axis=1
    ).astype(np.float32)  # (S, 64)
    tri = np.tril(np.ones((P, P), np.float16))

    in_maps = []
    for c in range(N_CORES):
        b, g = divmod(c, 4)
        wqkv = np.concatenate(
            [
                w_q[:, g * 256 : (g + 1) * 256],
                w_k[:, g * 64 : (g + 1) * 64],
                w_v[:, g * 64 : (g + 1) * 64],
            ],
            axis=1,
        ).astype(np.float32)
        in_maps.append(
            {
                "xT": np.ascontiguousarray(x[b].T),
                "wqkv": np.ascontiguousarray(wqkv),
                "wo": np.ascontiguousarray(
                    w_o[g * 256 : (g + 1) * 256, :]
                ).astype(np.float16),
                "cs": cs,
                "tri": tri,
            }
        )
    return in_maps


def kernel(x, w_q, w_k, w_v, w_o):
    from concourse.bass_utils import run_bass_kernel_spmd

    nc = get_nc()
    in_maps = make_in_maps(x, w_q, w_k, w_v, w_o)
    res = run_bass_kernel_spmd(nc, in_maps, list(range(N_CORES))).results
    y = np.zeros((2, SEQ, D_MODEL), np.float32)
    for c in range(N_CORES):
        y[c // 4] += res[c]["y"]
    return y
